# revision 4
# baseline (speedup 1.0000x reference)
"""Single-head causal attention (B=8, T=2048, C=1024, head_dim=64) on 8 TRN2 NeuronCores.

Sharding: data-parallel over batch -- one batch element per core, qkv weights
replicated. Host prep per core: x[b] transposed to [C, T] fp16; W pre-packed into
the SBUF chunk layout (one contiguous DMA).

Device schedule (v2 -- PE kept gap-free to hold the 2.4GHz p-state):
  kqT = Wkq^T x^T   [128, T] PSUM accum (j-outer, paced by the xt chunk DMAs)
  vT  = Wv^T x^T    [64, T]
  per t-group g: Vector evacuates kq/v PSUM group g (fused bias add + fp16
  cast), qT via SBUF-SBUF partition-shift DMA, v1 [s, hd] tiles via one
  grouped DMA-transpose XBAR; then the attention pairs for group g:
  ST pair -> exp on Scalar (Scalar does ONLY exp) -> diag-mask on Vector ->
  PV accumulate into acc_g [65, 512] (row 64 = denominator via ones column).
  epilogue: acc_g copied PSUM->SBUF (Vector) and DMA'd out UNNORMALIZED as
  [4, 65, 512] f32; host does the divide + transpose (no device transposes).
"""

import numpy as np

import concourse.bass as bass
import concourse.mybir as mybir
from concourse import bacc
from concourse.bass import ts
from concourse.bass_utils import run_bass_kernel_spmd
from concourse.tile import TileContext

B, T, C = 8, 2048, 1024
HD = 64
N_CORES = 8
NJ = C // 128  # contraction chunks for the qkv projection
NT = T // 128  # 128-row tiles along T
NG = T // 512  # 512-col groups along T
FP16 = mybir.dt.float16
CST_W = 8 * 192 + 2 + 128 + 64 + 128 + 128  # 1986 (layout kept from v1)
F32 = mybir.dt.float32
EXP = mybir.ActivationFunctionType.Exp


def build_nc() -> bass.Bass:
    nc = bacc.Bacc(None, target_bir_lowering=False)
    # w is pre-packed on host: [128, NJ*192] with w[p, j*192+m] = W[j*128+p, m]
    xt = nc.declare_dram_parameter("xt", [C, T], FP16, isOutput=False)
    # cst packs, per partition: NJ*192 w-chunk cols | bkq | bv | msk | (unused)
    cst = nc.declare_dram_parameter("cst", [128, CST_W], FP16, isOutput=False)
    # unnormalized output: per 512-col group, rows 0:64 = sum(P v), row 64 = sum(P)
    out = nc.declare_dram_parameter("out", [NG, HD + 1, 512], F32, isOutput=True)

    with TileContext(nc) as tc:
        with (
            tc.tile_pool(name="consts", bufs=1) as consts,
            tc.tile_pool(name="xtp", bufs=NJ) as xtp,
            tc.tile_pool(name="kqv", bufs=1) as kqv,
            tc.tile_pool(name="ptp", bufs=3) as ptp,
            tc.tile_pool(name="osb", bufs=2) as osb,
        ):
            # --- constants: one contiguous DMA ---
            cst_sb = consts.tile([128, CST_W], FP16)
            w_sb = cst_sb  # cols j*192 + [0:128) = Wkq_j, + [128:192) = Wv_j
            msk_sb = cst_sb[:, 1538:1666]
            wu_sb = consts.tile([1, 512], FP16)
            nc.vector.memset(wu_sb[:], 1.0)
            bias32 = consts.tile([128, 2], F32)

            kqT = kqv.tile([128, T], FP16)
            qT = kqv.tile([64, T], FP16)
            vT = kqv.tile([64, T], FP16)
            v1 = kqv.tile([128, NT, 80], FP16)  # [s, hd | ones | pad] per t-tile
            nc.vector.memset(v1[:, :, HD:HD + 1], 1.0)

            # --- DMAs: chunk 0 + constants first, then remaining chunks ---
            xts = []
            for j in range(NJ):
                xt_t = xtp.tile([128, T], FP16, tag="xt")
                eng = nc.sync if j % 2 == 0 else nc.scalar
                eng.dma_start(out=xt_t[:], in_=xt[ts(j, 128), :])
                xts.append(xt_t)
                if j == 0:
                    nc.scalar.dma_start(out=cst_sb[:], in_=cst[:, :])
                    nc.vector.tensor_copy(bias32[:, 0:1], cst_sb[:, 1536:1537])
                    nc.vector.tensor_copy(bias32[0:64, 1:2], cst_sb[0:64, 1537:1538])

            with tc.tile_pool(name="psp", bufs=8, space=bass.MemorySpace.PSUM) as psp:
                # warmups bridge the DMA lead-in so the PE p-state ramp starts
                # early and the stream has no gap before the first kq matmul
                wu_ps = psp.tile([128, 512], F32, tag="p", name="wu_ps")
                for r in range(3):
                    nc.tensor.matmul(wu_ps[:], wu_sb[:, 0:128], wu_sb[:], start=True, stop=True)
                kq_accs = [psp.tile([128, 512], F32, tag="p", name=f"kq_acc{n}") for n in range(NG)]
                v_accs = [psp.tile([64, 512], F32, tag="p", name=f"v_acc{n}") for n in range(NG)]
                for j in range(NJ):
                    first, last = j == 0, j == NJ - 1
                    for n in range(NG):
                        nc.tensor.matmul(
                            kq_accs[n][:], w_sb[:, j * 192:j * 192 + 128], xts[j][:, ts(n, 512)],
                            start=first, stop=last,
                        )
                    for n in range(NG):
                        nc.tensor.matmul(
                            v_accs[n][:], w_sb[:, j * 192 + 128:j * 192 + 192], xts[j][:, ts(n, 512)],
                            start=first, stop=last,
                        )
                # boundary: evacuate all groups on Vector (fused bias + cast),
                # q partition-shift + v1 transpose DMAs on sync; PE warmups
                # cover the group-0 evac/shift latency
                for n in range(NG):
                    nc.vector.tensor_scalar_add(
                        kqT[:, ts(n, 512)], kq_accs[n][:], bias32[:, 0:1])
                    nc.sync.dma_start(out=qT[:, ts(n, 512)], in_=kqT[64:128, ts(n, 512)])
                    nc.vector.tensor_scalar_add(
                        vT[:, ts(n, 512)], v_accs[n][:], bias32[0:64, 1:2])
                    nc.sync.dma_start(
                        out=v1[:, 4 * n:4 * n + 4, 0:HD], in_=vT[:, ts(n, 512)],
                        transpose=True,
                    )
                for r in range(6):
                    nc.tensor.matmul(wu_ps[:], wu_sb[:, 0:128], wu_sb[:], start=True, stop=True)

            # --- attention, t-group outer: ST pieces for two s-chunks share a
            # [128,1024] PSUM tile and one exp (Scalar does ONLY exp); diag
            # masks on GpSimd; PV accumulates [65, 512], denominator in row 64 ---
            with (
                tc.tile_pool(name="pso", bufs=2, space=bass.MemorySpace.PSUM) as pso,
                tc.tile_pool(name="pst", bufs=3, space=bass.MemorySpace.PSUM) as pst,
            ):
                def do_pair(g, p, acc):
                    gb = 512 * g
                    jmax = 4 * g + 3
                    jA, jB = 2 * p, 2 * p + 1
                    aA, aB = max(128 * jA, gb), max(128 * jB, gb)
                    stp = pst.tile([128, 1024], F32, tag="st", name=f"stp_{g}_{p}")
                    ptt = ptp.tile([128, 1024], FP16, tag="pt", name=f"ptt_{g}_{p}")
                    for jj, a, col in ((jA, aA, 0), (jB, aB, 512)):
                        nc.tensor.matmul(
                            stp[:, col + a - gb:col + 512],
                            kqT[0:64, ts(jj, 128)], qT[:, a:gb + 512],
                            start=True, stop=True,
                        )
                    if jB >= 4 * g:
                        for jj, a, col in ((jA, aA, 0), (jB, aB, 512)):
                            nc.scalar.activation(
                                ptt[:, col + a - gb:col + 512],
                                stp[:, col + a - gb:col + 512], EXP, scale=0.125,
                            )
                    else:
                        nc.scalar.activation(ptt[:], stp[:], EXP, scale=0.125)
                    for jj, a, col in ((jA, aA, 0), (jB, aB, 512)):
                        if jj >= 4 * g:
                            nc.gpsimd.tensor_mul(
                                ptt[:, col + a - gb:col + a - gb + 128],
                                ptt[:, col + a - gb:col + a - gb + 128], msk_sb,
                            )
                        nc.tensor.matmul(
                            acc[:, a - gb:512], v1[:, jj, 0:65],
                            ptt[:, col + a - gb:col + 512],
                            start=(jj == 0), stop=(jj == jmax),
                        )

                for g in range(NG):
                    acc = pso.tile([65, 512], F32, tag="o", name=f"outT_acc{g}")
                    for p in range(2 * g + 2):
                        do_pair(g, p, acc)
                    ob = osb.tile([65, 512], F32, tag="ob", name=f"ob{g}")
                    nc.vector.tensor_copy(ob[:], acc[:])
                    nc.gpsimd.dma_start(out=out[g], in_=ob[:])
    nc.compile()
    return nc


_NC_CACHE = None


def _get_nc() -> bass.Bass:
    global _NC_CACHE
    if _NC_CACHE is None:
        _NC_CACHE = build_nc()
    return _NC_CACHE


def make_in_maps(x: np.ndarray, W: np.ndarray, b: np.ndarray) -> list[dict]:
    cst = np.zeros((128, CST_W), dtype=np.float16)
    # w chunks: cst[p, j*192+m] = W[j*128+p, m]
    cst[:, :NJ * 3 * HD] = (
        W.astype(np.float16).reshape(NJ, 128, 3 * HD).transpose(1, 0, 2).reshape(128, NJ * 3 * HD)
    )
    cst[:, 1536] = b[0:128].astype(np.float16)
    cst[0:64, 1537] = b[128:192].astype(np.float16)
    cst[:, 1538:1666] = np.triu(np.ones((128, 128), dtype=np.float16))  # keep s <= t
    cst = np.ascontiguousarray(cst)
    in_maps = []
    for core in range(N_CORES):
        xtc = np.ascontiguousarray(x[core].astype(np.float16).T)
        in_maps.append({"xt": xtc, "cst": cst})
    return in_maps


def run(x, W, b, trace: bool = False):
    """Returns (output [B, T, HD] fp32, BassKernelResults)."""
    x, W, b = np.asarray(x), np.asarray(W), np.asarray(b)
    nc = _get_nc()
    res = run_bass_kernel_spmd(nc, make_in_maps(x, W, b), list(range(N_CORES)), trace=trace)
    outs = []
    for i in range(N_CORES):
        o = res.results[i]["out"]  # [NG, 65, 512] unnormalized, transposed
        y = (o[:, 0:HD, :] / o[:, HD:HD + 1, :]).transpose(0, 2, 1).reshape(T, HD)
        outs.append(y)
    return np.stack(outs, axis=0).astype(np.float32), res


def kernel(x, W, b) -> np.ndarray:
    out, _ = run(x, W, b)
    return out


# revision 12
# speedup vs baseline: 1.2726x; 1.2726x over previous
"""Single-head causal attention (B=8, T=2048, C=1024, head_dim=64) on 8 TRN2 NeuronCores.

Sharding: data-parallel over batch -- one batch element per core, qkv weights
replicated. Host prep per core: x[b] transposed to [C, T] fp16; W pre-packed into
the SBUF chunk layout (one contiguous DMA).

Device schedule (v2 -- PE kept gap-free to hold the 2.4GHz p-state):
  kqT = Wkq^T x^T   [128, T] PSUM accum (j-outer, paced by the xt chunk DMAs)
  vT  = Wv^T x^T    [64, T]
  per t-group g: Vector evacuates kq/v PSUM group g (fused bias add + fp16
  cast), qT via SBUF-SBUF partition-shift DMA, v1 [s, hd] tiles via one
  grouped DMA-transpose XBAR; then the attention pairs for group g:
  ST pair -> exp on Scalar (Scalar does ONLY exp) -> diag-mask on Vector ->
  PV accumulate into acc_g [65, 512] (row 64 = denominator via ones column).
  epilogue: acc_g copied PSUM->SBUF (Vector) and DMA'd out UNNORMALIZED as
  [4, 65, 512] f32; host does the divide + transpose (no device transposes).
"""

import numpy as np

import concourse.bass as bass
import concourse.mybir as mybir
from concourse import bacc
from concourse.bass import ts
from concourse.bass_utils import run_bass_kernel_spmd
from concourse.tile import TileContext

B, T, C = 8, 2048, 1024
HD = 64
N_CORES = 8
NJ = C // 128  # contraction chunks for the qkv projection
NT = T // 128  # 128-row tiles along T
NG = T // 512  # 512-col groups along T
FP16 = mybir.dt.float16
IDENT = mybir.ActivationFunctionType.Identity
CST_W = 8 * 192 + 2 + 128 + 64 + 128 + 128  # 1986 (layout kept from v1)
F32 = mybir.dt.float32
EXP = mybir.ActivationFunctionType.Exp


def build_nc() -> bass.Bass:
    nc = bacc.Bacc(None, target_bir_lowering=False)
    # w is pre-packed on host: [128, NJ*192] with w[p, j*192+m] = W[j*128+p, m]
    xt = nc.declare_dram_parameter("xt", [C, T], FP16, isOutput=False)
    # cst packs, per partition: NJ*192 w-chunk cols | bkq | bv | msk | (unused)
    cst = nc.declare_dram_parameter("cst", [128, CST_W], FP16, isOutput=False)
    # unnormalized output: per 512-col group, rows 0:64 = sum(P v), row 64 = sum(P)
    out = nc.declare_dram_parameter("out", [NG, HD + 1, 512], F32, isOutput=True)

    with TileContext(nc) as tc:
        with (
            tc.tile_pool(name="consts", bufs=1) as consts,
            tc.tile_pool(name="xtp", bufs=NJ) as xtp,
            tc.tile_pool(name="kqv", bufs=1) as kqv,
            tc.tile_pool(name="ptp", bufs=4) as ptp,
            tc.tile_pool(name="osb", bufs=2) as osb,
        ):
            # --- constants: one contiguous DMA ---
            cst_sb = consts.tile([128, CST_W], FP16)
            w_sb = cst_sb  # cols j*192 + [0:128) = Wkq_j, + [128:192) = Wv_j
            msk_sb = cst_sb[:, 1538:1666]
            wu_sb = consts.tile([1, 512], FP16)
            nc.vector.memset(wu_sb[:], 1.0)
            bias32 = consts.tile([128, 2], F32)

            kqT = kqv.tile([128, T], FP16)
            qT = kqv.tile([64, T], FP16)
            vT = kqv.tile([64, T], FP16)
            v1 = kqv.tile([128, NT, 80], FP16)  # [s, hd | ones | pad] per t-tile
            nc.vector.memset(v1[:, :, HD:HD + 1], 1.0)

            # --- DMAs: chunk 0 + constants first, then remaining chunks ---
            xts = []
            for j in range(NJ):
                xt_t = xtp.tile([128, T], FP16, tag="xt")
                eng = nc.sync if j % 2 == 0 else nc.scalar
                eng.dma_start(out=xt_t[:], in_=xt[ts(j, 128), :])
                xts.append(xt_t)
                if j == 0:
                    nc.scalar.dma_start(out=cst_sb[:], in_=cst[:, :])
                    nc.vector.tensor_copy(bias32[:, 0:1], cst_sb[:, 1536:1537])
                    nc.vector.tensor_copy(bias32[0:64, 1:2], cst_sb[0:64, 1537:1538])

            with tc.tile_pool(name="psp", bufs=8, space=bass.MemorySpace.PSUM) as psp:
                # warmups bridge the DMA lead-in so the PE p-state ramp starts
                # early and the stream has no gap before the first kq matmul
                wu_ps = psp.tile([128, 512], F32, tag="p", name="wu_ps")
                for r in range(5):
                    nc.tensor.matmul(wu_ps[:], wu_sb[:, 0:128], wu_sb[:], start=True, stop=True)
                kq_accs = [psp.tile([128, 512], F32, tag="p", name=f"kq_acc{n}") for n in range(NG)]
                v_accs = [psp.tile([64, 512], F32, tag="p", name=f"v_acc{n}") for n in range(NG)]
                for j in range(NJ):
                    first, last = j == 0, j == NJ - 1
                    for n in range(NG):
                        nc.tensor.matmul(
                            kq_accs[n][:], w_sb[:, j * 192:j * 192 + 128], xts[j][:, ts(n, 512)],
                            start=first, stop=last,
                        )
                    for n in range(NG):
                        nc.tensor.matmul(
                            v_accs[n][:], w_sb[:, j * 192 + 128:j * 192 + 192], xts[j][:, ts(n, 512)],
                            start=first, stop=last,
                        )
                # boundary: evacuate all groups (fused bias + cast) split
                # across Vector (groups 0-1) and GpSimd (groups 2-3) so they
                # finish in ~half the serial time; q partition-shift + v1
                # transpose DMAs on sync; PE warmups cover the latency
                for n in range(2):
                    nc.vector.tensor_scalar_add(
                        kqT[:, ts(n, 512)], kq_accs[n][:], bias32[:, 0:1])
                    nc.sync.dma_start(out=qT[:, ts(n, 512)], in_=kqT[64:128, ts(n, 512)])
                    nc.vector.tensor_scalar_add(
                        vT[:, ts(n, 512)], v_accs[n][:], bias32[0:64, 1:2])
                    nc.sync.dma_start(
                        out=v1[:, 4 * n:4 * n + 4, 0:HD], in_=vT[:, ts(n, 512)],
                        transpose=True,
                    )
                for n in range(2, NG):
                    nc.scalar.activation(
                        kqT[:, ts(n, 512)], kq_accs[n][:], IDENT, bias=bias32[:, 0:1])
                    nc.sync.dma_start(out=qT[:, ts(n, 512)], in_=kqT[64:128, ts(n, 512)])
                    nc.scalar.activation(
                        vT[:, ts(n, 512)], v_accs[n][:], IDENT, bias=bias32[0:64, 1:2])
                    nc.sync.dma_start(
                        out=v1[:, 4 * n:4 * n + 4, 0:HD], in_=vT[:, ts(n, 512)],
                        transpose=True,
                    )
                for r in range(8):
                    nc.tensor.matmul(wu_ps[:], wu_sb[:, 0:128], wu_sb[:], start=True, stop=True)

            # --- attention, t-group outer: ST pieces for two s-chunks share a
            # [128,1024] PSUM tile and one exp (Scalar does ONLY exp); diag
            # masks on GpSimd; PV accumulates [65, 512], denominator in row 64 ---
            with (
                tc.tile_pool(name="pst", bufs=3, space=bass.MemorySpace.PSUM) as pst,
                tc.tile_pool(name="pso", bufs=2, space=bass.MemorySpace.PSUM) as pso,
            ):
                def do_pair(g, p, acc):
                    gb = 512 * g
                    jmax = 4 * g + 3
                    jA, jB = 2 * p, 2 * p + 1
                    aA, aB = max(128 * jA, gb), max(128 * jB, gb)
                    stp = pst.tile([128, 1024], F32, tag="st", name=f"stp_{g}_{p}")
                    ptt = ptp.tile([128, 1024], FP16, tag="pt", name=f"ptt_{g}_{p}")
                    for jj, a, col in ((jA, aA, 0), (jB, aB, 512)):
                        nc.tensor.matmul(
                            stp[:, col + a - gb:col + 512],
                            kqT[0:64, ts(jj, 128)], qT[:, a:gb + 512],
                            start=True, stop=True,
                        )
                    if jB >= 4 * g:
                        for jj, a, col in ((jA, aA, 0), (jB, aB, 512)):
                            nc.scalar.activation(
                                ptt[:, col + a - gb:col + 512],
                                stp[:, col + a - gb:col + 512], EXP, scale=0.125,
                            )
                    else:
                        nc.scalar.activation(ptt[:], stp[:], EXP, scale=0.125)
                    for jj, a, col in ((jA, aA, 0), (jB, aB, 512)):
                        if jj >= 4 * g:
                            nc.vector.tensor_mul(
                                ptt[:, col + a - gb:col + a - gb + 128],
                                ptt[:, col + a - gb:col + a - gb + 128], msk_sb,
                            )
                        nc.tensor.matmul(
                            acc[:, a - gb:512], v1[:, jj, 0:65],
                            ptt[:, col + a - gb:col + 512],
                            start=(jj == 0), stop=(jj == jmax),
                        )

                for g in range(NG):
                    acc = pso.tile([65, 512], F32, tag="o", name=f"outT_acc{g}")
                    for p in range(2 * g + 2):
                        do_pair(g, p, acc)
                    ob = osb.tile([65, 512], F32, tag="ob", name=f"ob{g}")
                    nc.vector.tensor_copy(ob[:], acc[:])
                    nc.sync.dma_start(out=out[g], in_=ob[:])
    nc.compile()
    return nc


_NC_CACHE = None


def _get_nc() -> bass.Bass:
    global _NC_CACHE
    if _NC_CACHE is None:
        _NC_CACHE = build_nc()
    return _NC_CACHE


def make_in_maps(x: np.ndarray, W: np.ndarray, b: np.ndarray) -> list[dict]:
    cst = np.zeros((128, CST_W), dtype=np.float16)
    # w chunks: cst[p, j*192+m] = W[j*128+p, m]
    cst[:, :NJ * 3 * HD] = (
        W.astype(np.float16).reshape(NJ, 128, 3 * HD).transpose(1, 0, 2).reshape(128, NJ * 3 * HD)
    )
    cst[:, 1536] = b[0:128].astype(np.float16)
    cst[0:64, 1537] = b[128:192].astype(np.float16)
    cst[:, 1538:1666] = np.triu(np.ones((128, 128), dtype=np.float16))  # keep s <= t
    cst = np.ascontiguousarray(cst)
    in_maps = []
    for core in range(N_CORES):
        xtc = np.ascontiguousarray(x[core].astype(np.float16).T)
        in_maps.append({"xt": xtc, "cst": cst})
    return in_maps


def run(x, W, b, trace: bool = False):
    """Returns (output [B, T, HD] fp32, BassKernelResults)."""
    x, W, b = np.asarray(x), np.asarray(W), np.asarray(b)
    nc = _get_nc()
    res = run_bass_kernel_spmd(nc, make_in_maps(x, W, b), list(range(N_CORES)), trace=trace)
    outs = []
    for i in range(N_CORES):
        o = res.results[i]["out"]  # [NG, 65, 512] unnormalized, transposed
        y = (o[:, 0:HD, :] / o[:, HD:HD + 1, :]).transpose(0, 2, 1).reshape(T, HD)
        outs.append(y)
    return np.stack(outs, axis=0).astype(np.float32), res


def kernel(x, W, b) -> np.ndarray:
    out, _ = run(x, W, b)
    return out
